# revision 12
# baseline (speedup 1.0000x reference)
"""ArcFace (AngularPenaltySMLoss) distributed Bass kernel for 8 TRN2 NeuronCores.

v4 strategy (vocab/tensor parallel, per sharding hint):
  - W [50000, 512] sharded along classes: core k owns [6250k, 6250(k+1)),
    padded to 6272 cols (pad logit 0; host subtracts the pad exps).
  - Host normalizes x rows during fp8 packing, so the exp argument is a
    CONSTANT scale of the fp8 matmul PSUM — no on-device norms, no
    per-partition scale APs, no Sqrt ACT-table switch.
  - PE: fp8e4 DoubleRow matmuls (512-col, K=256/instr) at the 157 TF/s
    roofline — 832 matmuls = the ~167us PE floor. LDWEIGHTS pipelines
    behind the matmul stream (never stalls it).
  - The exp+row-sum of all 4096 x 6272 logits is split across TWO
    engines so neither gates the PE:
      * ScalarE ACT Exp with fused accum_out (chunks 0, 2a, 2b, 4)
      * ONE custom DVE instruction per chunk (EXP8SUM_ANT):
          q = (QA*v + QB)*v + QC;  out = ((q^2)^2)^2 ~= exp(SC*v)
        with accum=add emitting the row-sum directly (8 ALU stages).
        The quadratic is a weighted minimax fit of e^(z/8); validated to
        ~1e-5 final loss error vs the exact reference.
  - PSUM is 4 rotating [128,1024] bufs; the 128-col runt chunk shrinks
    one wrap window to ~1.8us, so phase 2 orders chunks [0,1,2,3,6,5,4]
    to put the short windows on the faster-draining DVE, and chunk 2
    (ACT) is split into 2x512 ACTIVATEs to halve its drain latency.
  - DMA: each SBUF tile is one contiguous-per-partition DRAM region
    (2-4KB elements), one DMA per tile on the sync HWDGE ring, ordered
    by consumption; wt chunk 0 ships as 2x512-col tiles so the first
    matmul only waits for 1MB. Processing is column-major over j=0..3
    first (phase 1) so the wt stream only needs ~133 GB/s; then
    row-major j=4..31 (phase 2).
  - Target path: host pre-gathers W[target] rows (packing, like the
    transposes); device dots them with xn rows: 2x256-col muls + 4x128
    partial reduces per row-group, spread across j's, written to out
    cols 32..47 (host sums each group of 4).
  - Host combine: sum the 8 [128, 48] partials, subtract pad/target
    exps, arcface scalar tail, mean.
"""

import functools
import math
import sys

import numpy as np

sys.path.insert(0, "/opt/trn_rl_repo")

N, D, C = 4096, 512, 50000
NCORES = 8
CSH = C // NCORES          # 6250 classes per core
CPAD = 6272                # 49*128
S = 30.0
MARG = 0.4
EPS = 1e-7
SX = 512.0                 # fp8 scale for normalized x
SW = 512.0                 # fp8 scale for W
SC = S / (SX * SW)         # exp(SC * psum) == exp(S * cos)
# q(z) = A2 z^2 + A1 z + A0 fit so q^8 ~ e^z under z~N(0,0.62) weighting
A2, A1, A0 = 0.00852011, 0.12491175, 0.99982349
QA = A2 * SC * SC
QB = A1 * SC
QC = A0
PAD_VAL = A0 ** 8          # DVE runt chunk holds the pads: q(0)^8 per pad
PADS_TOTAL = float((CPAD - CSH) * NCORES) * PAD_VAL
ROWS_PER_CORE = N // NCORES                 # 512
NTILES = N // 128                           # 32
# chunks per row-tile: 6x1024 + 128 (runt)
CHUNKS = [(i * 1024, 1024) for i in range(6)] + [(6144, 128)]
# SSG column per (chunk, half): c2 is split into two 512 halves
SSGCOL = {(0, 0): 0, (1, 0): 1, (2, 0): 2, (2, 1): 3, (3, 0): 4,
          (4, 0): 5, (5, 0): 6, (6, 0): 7}
NSG = 8
JPH1 = 4                   # phase-1 row-tiles (column-major while wt streams)


def _register_exp8():
    """Register the EXP8SUM_ANT custom DVE op (idempotent)."""
    from operator import add as _add

    from concourse import dve_ops
    from concourse.dve_spec import C0, C1, C2, Spec, Src0, lower, sq
    from concourse.dve_uop import DveOpSpec

    name = "EXP8SUM_ANT"
    if name in dve_ops._SUB_OPCODE_FOR_NAME:
        return next(op for op in dve_ops.OPS if op.name == name)

    body = sq(sq(sq((Src0 * C0 + C1) * Src0 + C2)))

    def _ref(in0, in1, s0, s1, imm2):
        q = (
            (np.float32(s0) * in0.astype(np.float32) + np.float32(s1)) * in0
            + np.float32(imm2)
        ).astype(np.float32)
        q = (q * q).astype(np.float32)
        q = (q * q).astype(np.float32)
        q = (q * q).astype(np.float32)
        return q, q.reshape(q.shape[0], -1).sum(axis=-1, keepdims=True).astype(
            np.float32
        )

    spec = Spec(body=body, accum=_add, reference=_ref)
    row = dve_ops._CUSTOM_DVE_ROW_BASE + len(dve_ops.OPS)
    shas = {}
    for ver in ("v3", "v4"):
        s = DveOpSpec(name=name, opcode=row, uops=lower(spec, ver=ver), rd1_en=False)
        shas[ver] = s.sha(ver)
    op = dve_ops.DveOp(name, spec, subdim=False, uops_sha=shas)
    dve_ops.OPS.append(op)
    dve_ops._SUB_OPCODE_FOR_NAME[name] = row
    dve_ops.CUSTOM_DVE_SPECS[name] = spec
    return op


def build_graph():
    from concourse import bacc, bass, mybir, tile

    exp8 = _register_exp8()

    f32 = mybir.dt.float32
    bf16 = mybir.dt.bfloat16
    f8 = mybir.dt.float8e4
    AF = mybir.ActivationFunctionType
    ALU = mybir.AluOpType

    nc = bacc.Bacc(
        "TRN2",
        target_bir_lowering=False,
        debug=False,
        enable_asserts=False,
        num_devices=NCORES,
    )

    # per-tile contiguous layouts: one DMA per SBUF tile, 2-4KB elements.
    # wt chunk 0 ships as two 512-col tiles (w0a/w0b) for a faster first mm.
    xt_d = nc.dram_tensor("xt", [8, 128, 2, 2, 512], f8, kind="ExternalInput")
    w0_d = nc.dram_tensor("w0", [2, 128, 2, 2, 512], f8, kind="ExternalInput")
    wt_d = nc.dram_tensor("wt", [5, 128, 2, 2, 1024], f8, kind="ExternalInput")
    wtr_d = nc.dram_tensor("wtr", [128, 2, 2, 128], f8, kind="ExternalInput")
    xo_d = nc.dram_tensor("xo", [ROWS_PER_CORE, D], f32, kind="ExternalInput")
    wg_d = nc.dram_tensor("wg", [ROWS_PER_CORE, D], f32, kind="ExternalInput")
    out_d = nc.dram_tensor("out", [128, 48], f32, kind="ExternalOutput")

    with tile.TileContext(nc) as tc:
        with (
            tc.tile_pool(name="big", bufs=1) as bigp,
            tc.tile_pool(name="wk", bufs=3) as wk,
            tc.tile_pool(name="ps", bufs=4, space="PSUM") as pp,
        ):
            w0_sb = [
                bigp.tile([128, 2, 2, 512], f8, name=f"w0sb{h}", tag=f"w0sb{h}")
                for h in range(2)
            ]
            wt_sb = [
                bigp.tile([128, 2, 2, 1024], f8, name=f"wtsb{c}", tag=f"wtsb{c}")
                for c in range(1, 6)
            ]
            wtr_sb = bigp.tile([128, 2, 2, 128], f8, name="wtrsb", tag="wtrsb")
            xt_sb = [
                bigp.tile([128, 2, 2, 512], f8, name=f"xtsb{t}", tag=f"xtsb{t}")
                for t in range(8)
            ]

            # DMA order == consumption order; xo/wg queue after wt (needed
            # only from j=6, and they'd contend for HBM).
            nc.sync.dma_start(xt_sb[0][:], xt_d.ap()[0])
            for h in range(2):
                nc.sync.dma_start(w0_sb[h][:], w0_d.ap()[h])
            for c in range(1, 6):
                nc.sync.dma_start(wt_sb[c - 1][:], wt_d.ap()[c - 1])
            nc.sync.dma_start(wtr_sb[:], wtr_d.ap()[:])
            for t in range(1, 8):
                nc.sync.dma_start(xt_sb[t][:], xt_d.ap()[t])

            xo_sb = bigp.tile([128, 4, D], f32, name="xo_sb")
            wg_sb = bigp.tile([128, 4, D], f32, name="wg_sb")
            for jj in range(4):
                nc.sync.dma_start(
                    xo_sb[:, jj, :], xo_d.ap()[jj * 128:(jj + 1) * 128, :]
                )
                nc.sync.dma_start(
                    wg_sb[:, jj, :], wg_d.ap()[jj * 128:(jj + 1) * 128, :]
                )

            SSG = bigp.tile([128, NTILES, NSG], f32, name="SSG")
            CONTRIB = bigp.tile([128, 48], f32, name="CONTRIB")
            TD = [
                bigp.tile([128, D], f32, name=f"td{jj}", tag=f"td{jj}")
                for jj in range(4)
            ]

            # warmup: force the exp ACT-table load at t~0 (during DMA wait)
            warm = bigp.tile([128, 1], f32, name="warm")
            wsink = bigp.tile([128, 1], bf16, name="wsink")
            nc.vector.memset(warm[:], 0.0)
            nc.scalar.activation(wsink[:], warm[:], AF.Exp)

            def rhs_ap(c, cc, ncol):
                if c == 0:
                    return w0_sb[cc][:, :, :, 0:ncol]
                if c == 6:
                    return wtr_sb[:, :, :, 0:ncol]
                return wt_sb[c - 1][:, :, :, cc * 512:cc * 512 + ncol]

            def do_chunk(j, c, to_act, split=False):
                c0, w = CHUNKS[c]
                xoff = (j % 4) * 128
                pg = pp.tile([128, 1024], f32, name="pg", tag="pg")
                nhalf = (w + 511) // 512
                # cc-major matmul order so the first half's PSUM closes
                # early for split consumers
                for cc in range(nhalf):
                    ncol = min(512, w - cc * 512)
                    for g2 in range(2):
                        nc.tensor.matmul(
                            out=pg[:, cc * 512:cc * 512 + ncol],
                            lhsT=xt_sb[j // 4][:, g2, :, xoff:xoff + 128],
                            rhs=rhs_ap(c, cc, ncol)[:, g2],
                            start=(g2 == 0),
                            stop=(g2 == 1),
                            perf_mode=mybir.MatmulPerfMode.DoubleRow,
                        )
                halves = (
                    [(0, w, 0)] if not split else
                    [(h * 512, 512, h) for h in range(nhalf)]
                )
                for off, hw_, h in halves:
                    col = SSG[:, j, SSGCOL[(c, h)]:SSGCOL[(c, h)] + 1]
                    if to_act:
                        esink = wk.tile(
                            [128, 1024], bf16, name="esink", tag="esink"
                        )
                        nc.scalar.activation(
                            out=esink[:, off:off + hw_],
                            in_=pg[:, off:off + hw_],
                            func=AF.Exp,
                            scale=SC,
                            accum_out=col,
                        )
                    else:
                        scr = wk.tile([128, 1024], f32, name="scr", tag="scr")
                        nc.vector._custom_dve(
                            exp8,
                            out=scr[:, off:off + hw_],
                            in0=pg[:, off:off + hw_],
                            s0=QA,
                            s1=QB,
                            imm2=QC,
                            accum_out=col,
                        )

            # phase 1: column-major over j=0..JPH1-1 while wt streams in.
            # (j+c) parity keeps ACT/DVE interleaved within each c-pass; the
            # runt (c=6) always goes to DVE so the host pad term is uniform.
            for c in range(7):
                for j in range(JPH1):
                    to_act = ((c + j) % 2 == 0) if c < 6 else False
                    do_chunk(j, c, to_act, split=(c == 2))

            # phase 2: row-major, chunk order [0,1,2,3,6,5,4] so the short
            # wrap windows created by the runt land on DVE drains; c2 (ACT)
            # splits into 2x512 to halve its drain latency.  The 4 target
            # dots interleave as small DVE pieces at row-tile boundaries:
            # at j=6+4jj a 2x256 mul pair, at j+1/j+2 four 128-col partial
            # reduces into CONTRIB[:, 32+4jj .. ] (host sums each 4).
            P2 = [(0, True, False), (1, False, False), (2, True, True),
                  (3, False, False), (6, False, False), (5, False, False),
                  (4, True, False)]
            for j in range(JPH1, NTILES):
                if j >= 6 and (j - 6) % 4 == 0 and j <= 18:
                    jj = (j - 6) // 4
                    for h in range(2):
                        nc.vector.tensor_mul(
                            TD[jj][:, h * 256:(h + 1) * 256],
                            xo_sb[:, jj, h * 256:(h + 1) * 256],
                            wg_sb[:, jj, h * 256:(h + 1) * 256],
                        )
                if j >= 7 and (j - 7) % 4 in (0, 1) and j <= 20:
                    jj = (j - 7) // 4
                    qbase = 2 * ((j - 7) % 4)
                    for q in (qbase, qbase + 1):
                        nc.vector.tensor_reduce(
                            CONTRIB[:, 32 + 4 * jj + q:33 + 4 * jj + q],
                            TD[jj][:, q * 128:(q + 1) * 128],
                            mybir.AxisListType.X,
                            ALU.add,
                        )
                for c, to_act, split in P2:
                    do_chunk(j, c, to_act, split=split)

            # fold the 32x8 chunk sums -> per-row-tile sums, one DVE op
            nc.vector.tensor_reduce(
                CONTRIB[:, 0:32], SSG[:], mybir.AxisListType.X, ALU.add
            )
            nc.sync.dma_start(out_d.ap()[:, :], CONTRIB[:])

    nc.compile()
    return nc


@functools.lru_cache(maxsize=1)
def _compiled():
    return build_graph()


def _prep_in_maps(x, W, target):
    import ml_dtypes

    f8 = ml_dtypes.float8_e4m3fn
    x = np.asarray(x, dtype=np.float32)
    W = np.asarray(W, dtype=np.float32)
    target = np.asarray(target, dtype=np.int32)

    xn = x / np.linalg.norm(x, axis=1, keepdims=True)
    # xt[t, p, g, i, col] = xn[512t+col, (2g+i)*128 + p] * SX
    xv = np.clip(xn.T * SX, -240, 240).reshape(2, 2, 128, N)  # [g, i, p, n]
    xt = np.ascontiguousarray(
        xv.reshape(2, 2, 128, 8, 512).transpose(3, 2, 0, 1, 4)
    ).astype(f8)
    in_maps = []
    for k in range(NCORES):
        wtp = np.zeros((D, CPAD), dtype=np.float32)
        wtp[:, :CSH] = W[k * CSH:(k + 1) * CSH].T * SW
        wv = np.clip(wtp, -240, 240).reshape(2, 2, 128, CPAD)  # [g, i, p, c]
        w0 = np.ascontiguousarray(
            wv[:, :, :, :1024].reshape(2, 2, 128, 2, 512).transpose(3, 2, 0, 1, 4)
        ).astype(f8)
        wt = np.ascontiguousarray(
            wv[:, :, :, 1024:6144]
            .reshape(2, 2, 128, 5, 1024)
            .transpose(3, 2, 0, 1, 4)
        ).astype(f8)
        wtr = np.ascontiguousarray(
            wv[:, :, :, 6144:].transpose(2, 0, 1, 3)
        ).astype(f8)
        rows = slice(k * ROWS_PER_CORE, (k + 1) * ROWS_PER_CORE)
        in_maps.append(
            {
                "xt": xt,
                "w0": w0,
                "wt": wt,
                "wtr": wtr,
                "xo": np.ascontiguousarray(xn[rows]),
                "wg": np.ascontiguousarray(W[target[rows]]),
            }
        )
    return in_maps


def _combine(parts):
    """Host-side all-reduce of the per-core [128, 48] partials + scalar tail."""
    fs = np.zeros((128, 32), dtype=np.float64)
    tg = np.zeros(N, dtype=np.float64)
    for k, p in enumerate(parts):
        p = np.asarray(p, dtype=np.float64)
        fs += p[:, 0:32]
        # core k's target-cos for rows [512k, 512(k+1)): cols 32+4jj..35+4jj
        # hold 4 partial dots for rows n = 512k+128jj+p
        td = p[:, 32:48].reshape(128, 4, 4).sum(axis=2)  # [p, jj]
        tg[ROWS_PER_CORE * k:ROWS_PER_CORE * (k + 1)] = td.T.reshape(-1)
    # fs[p, col] <-> row n = 128*col + p
    full_sum = fs.T.reshape(-1)  # [4096]
    tcl = np.clip(tg, -1.0 + EPS, 1.0 - EPS)
    num = S * (tcl * math.cos(MARG) - np.sqrt(1.0 - tcl * tcl) * math.sin(MARG))
    excl = full_sum - PADS_TOTAL - np.exp(S * tg)
    denom = np.exp(num) + excl
    L = num - np.log(denom)
    return np.float32(-np.mean(L))


def kernel_run(x, W, target, trace=False, **kw):
    """Returns (loss_scalar, BassKernelResults)."""
    from concourse import bass_utils

    nc = _compiled()
    in_maps = _prep_in_maps(x, W, target)
    res = bass_utils.run_bass_kernel_spmd(
        nc, in_maps, core_ids=list(range(NCORES)), trace=trace, **kw
    )
    loss = _combine([r["out"] for r in res.results])
    return np.asarray(loss, dtype=np.float32), res


def kernel(x, W, target):
    loss, _ = kernel_run(x, W, target, trace=False)
    return loss


if __name__ == "__main__":
    nc = build_graph()
    print("graph built + compiled OK")
